# revision 15
# baseline (speedup 1.0000x reference)
"""MoE FFN (expert-parallel, host-routed) Trainium2 kernel.

Strategy: the gate depends only on inputs (x, Wg, bg), so routing is
part of the sharding decision: the host computes top-2 routing, compacts
each expert's tokens into a padded [CAP, C] block, and core e runs a
dense FFN (relu(x@W1+b1)@W2, bf16 matmuls / f32 accumulate) over expert
e's block. The host unshard step scatters each expert's rows back to
token order with the renormalized gate weights and adds the w-weighted
b2 terms (a 2-term axpy per token — the "psum" combine).

Device kernel layout per 256-token chunk (h-outer, W1/W2 interleaved so
the PE never idles): ps1[h] = sum_k W1[k,h].T @ xT[k]  (8 matmuls,
N=256) -> scalar-engine relu+b1 -> ht[h] bf16 -> ps2[tt,cc] +=
ht[h,tt].T @ W2[h,cc] (4 matmuls, N=512, accumulated over all 32 h).
W1/W2 stay SBUF-resident (128 KB/partition); PSUM: 4 banks ps2 + 2 ps1.
"""
import sys

sys.path.insert(0, "/opt/trn_rl_repo")

import numpy as np
import ml_dtypes

import bass_rust
import concourse.bass as bass
import concourse.mybir as mybir
import concourse.bass_utils as bu
from concourse.tile import TileContext

BF16 = ml_dtypes.bfloat16

B, T, C, E, H = 4, 2048, 1024, 8, 4096
NT = B * T          # 8192 tokens
P = 128
KC = C // P         # 8 k-tiles over C
KH = H // P         # 32 k-tiles over H

F32 = mybir.dt.float32
BF = mybir.dt.bfloat16
Relu = mybir.ActivationFunctionType.Relu


def _split_excess_waits(nc):
    """walrus codegen allows 1 sem-wait per instruction (2 on
    EventSemaphore). Move excess waits onto same-engine EventSemaphore
    insts placed just before (engine program order preserves semantics)."""
    for f in nc.m.functions:
        for bb in f.blocks:
            new = []
            changed = False
            for inst in bb.instructions:
                si = inst.sync_info
                cap = 2 if isinstance(inst, mybir.InstEventSemaphore) else 1
                if si is not None and len(si.on_wait) > cap:
                    waits = list(si.on_wait)
                    extra, keep = waits[:-cap], waits[-cap:]
                    for i in range(0, len(extra), 2):
                        w = mybir.InstEventSemaphore(
                            name=f"{inst.name}_presem{i}", ins=[], outs=[])
                        w.engine = inst.engine
                        w.sync_info = bass_rust.SyncInfo(
                            on_wait=extra[i:i + 2], on_update=[])
                        new.append(w)
                        changed = True
                    inst.sync_info = bass_rust.SyncInfo(
                        on_wait=keep, on_update=list(si.on_update))
                new.append(inst)
            if changed:
                bb.instructions = new


def _build_ffn(cap):
    """Dense FFN over cap pre-gathered tokens: y = relu(xT.T@W1 + b1)@W2."""
    nc = bass.Bass()
    xgt = nc.declare_dram_parameter("xgt", [C, cap], BF, isOutput=False)
    w1 = nc.declare_dram_parameter("w1", [C, H], BF, isOutput=False)
    b1c = nc.declare_dram_parameter("b1c", [P, KH], F32, isOutput=False)
    w2 = nc.declare_dram_parameter("w2", [H, C], BF, isOutput=False)
    out = nc.declare_dram_parameter("out", [cap, C], F32, isOutput=True)

    # chunk sizes: 256-token chunks with one (possibly ragged) tail
    sizes = []
    left = cap
    while left > 0:
        s = 256 if left >= 256 else left
        sizes.append(s)
        left -= s

    HH = H // 2  # W1 loaded as h-halves so chunk 0 starts sooner

    with TileContext(nc) as tc:
        with tc.tile_pool(name="wpool", bufs=1) as wpool, \
             tc.tile_pool(name="xpool", bufs=3) as xpool, \
             tc.tile_pool(name="hpool", bufs=18) as hpool, \
             tc.tile_pool(name="ypool", bufs=4) as ypool, \
             tc.tile_pool(name="ps1", bufs=2, space="PSUM") as ps1p, \
             tc.tile_pool(name="psd", bufs=1, space="PSUM") as psdp, \
             tc.tile_pool(name="ps2", bufs=1, space="PSUM") as ps2p:

            # DMA plan: each HWDGE queue sustains only ~200 GB/s, so split
            # every weight class across BOTH queues (even k/h on sync, odd
            # on scalar), ordered by first consumption: W1 h-first-half,
            # W1 h-second-half, then W2 in h order.
            b1c_sb = wpool.tile([P, KH], F32, tag="b1c")
            nc.sync.dma_start(out=b1c_sb[:], in_=b1c[:])
            xts_t = {}
            xgt_r = xgt.rearrange("(k p) t -> p k t", p=P)
            xts_t[0] = xpool.tile([P, KC, sizes[0]], BF,
                                  tag=f"xts{sizes[0]}", name="xts")
            nc.scalar.dma_start(out=xts_t[0][:], in_=xgt_r[:, :, 0:sizes[0]])

            def eng(i):
                return nc.sync if i % 2 == 0 else nc.scalar

            w1a_sb, w1b_sb = [], []
            for k in range(KC):
                ta = wpool.tile([P, HH], BF, tag=f"w1a{k}")
                eng(k).dma_start(out=ta[:], in_=w1[k * P:(k + 1) * P, 0:HH])
                w1a_sb.append(ta)
            for k in range(KC):
                tb = wpool.tile([P, HH], BF, tag=f"w1b{k}")
                eng(k).dma_start(out=tb[:], in_=w1[k * P:(k + 1) * P, HH:H])
                w1b_sb.append(tb)
            w2_sb = []
            for h in range(KH):
                t = wpool.tile([P, C], BF, tag=f"w2k{h}")
                eng(h).dma_start(out=t[:], in_=w2[h * P:(h + 1) * P, :])
                w2_sb.append(t)

            def w1_slice(k, h):
                if h < KH // 2:
                    return w1a_sb[k][:, h * P:(h + 1) * P]
                return w1b_sb[k][:, (h - KH // 2) * P:(h - KH // 2 + 1) * P]

            # HAM pre-warm: one long accumulation group of dummy matmuls
            # (no intermediate PSUM reads -> pure back-to-back PE stream)
            # keeps the PE busy from ~8us so the clock gate opens
            # (1.2 -> 2.4 GHz) before the real work arrives.
            NDUM = 30
            dsb = wpool.tile([P, 512], BF, tag="dummy")
            nc.vector.memset(dsb[:], 0.0)
            dps = psdp.tile([P, 512], F32, tag="dps", name="dps")
            for i in range(NDUM):
                nc.tensor.matmul(out=dps[:], lhsT=dsb[:, 0:P], rhs=dsb[:],
                                 start=(i == 0), stop=(i == NDUM - 1))

            c0 = 0
            for ci, tok in enumerate(sizes):
                ntt = -(-tok // P)
                # prefetch next chunk's x^T on the scalar queue
                if ci + 1 < len(sizes):
                    nt_ = sizes[ci + 1]
                    xts_t[ci + 1] = xpool.tile([P, KC, nt_], BF,
                                               tag=f"xts{nt_}", name="xts")
                    nc.scalar.dma_start(
                        out=xts_t[ci + 1][:],
                        in_=xgt_r[:, :, c0 + tok:c0 + tok + nt_])
                xts = xts_t.pop(ci)

                ps2 = [[ps2p.tile([P, 512], F32, tag=f"ps2_{tt}_{cc}",
                                  name=f"ps2_{tt}_{cc}")
                        for cc in range(2)] for tt in range(ntt)]
                ht = {}

                def w2_group(h):
                    for tt in range(ntt):
                        mm = min(P, tok - tt * P)
                        for cc in range(2):
                            nc.tensor.matmul(
                                out=ps2[tt][cc][0:mm, :],
                                lhsT=ht[h][:, tt * P:tt * P + mm],
                                rhs=w2_sb[h][:, cc * 512:(cc + 1) * 512],
                                start=(h == 0), stop=(h == KH - 1))

                for h in range(KH):
                    ps1 = ps1p.tile([P, 256], F32, tag="ps1",
                                    name="ps1")[:, 0:tok]
                    for k in range(KC):
                        nc.tensor.matmul(
                            out=ps1,
                            lhsT=w1_slice(k, h),
                            rhs=xts[:, k, :],
                            start=(k == 0), stop=(k == KC - 1))
                    htt = hpool.tile([P, 256], BF, tag="ht",
                                     name="ht")[:, 0:tok]
                    nc.scalar.activation(htt, ps1, Relu,
                                         bias=b1c_sb[:, h:h + 1])
                    ht[h] = htt
                    # W2 runs behind W1 so the relu latency is hidden by PE
                    # work; chunk 0 uses a deep lag so its W2 groups are
                    # reached only after the w2 h-tiles have streamed in.
                    lag = 15 if ci == 0 else 2
                    if h >= lag:
                        w2_group(h - lag)
                for h in range(KH - lag, KH):
                    w2_group(h)

                for tt in range(ntt):
                    mm = min(P, tok - tt * P)
                    y = ypool.tile([P, C], F32, tag="y", name="y")
                    # split the PSUM->SBUF drain across DVE and ACT so both
                    # halves land before the next chunk's first W2 group
                    nc.vector.tensor_copy(out=y[0:mm, 0:512],
                                          in_=ps2[tt][0][0:mm, :])
                    nc.scalar.activation(
                        y[0:mm, 512:1024], ps2[tt][1][0:mm, :],
                        mybir.ActivationFunctionType.Copy)
                    nc.gpsimd.dma_start(
                        out=out[c0 + tt * P:c0 + tt * P + mm, :],
                        in_=y[0:mm, :])
                c0 += tok

    import os
    if os.environ.get("NOSPLIT", "0") != "1":
        _split_excess_waits(nc)
    return nc


_NC_CACHE = {}


def _get_nc(cap):
    if cap not in _NC_CACHE:
        _NC_CACHE[cap] = _build_ffn(cap)
    return _NC_CACHE[cap]


def _route(x, Wg, bg):
    """Host top-2 routing with the exact ops the reference uses, so the
    selection bit-matches the oracle on whatever jax backend is active."""
    try:
        import jax
        import jax.numpy as jnp

        gs = jax.nn.softmax(
            jnp.einsum("btc,ce->bte", jnp.asarray(x), jnp.asarray(Wg))
            + jnp.asarray(bg), axis=-1)
        ts, ti = jax.lax.top_k(gs, 2)
        tp = jax.nn.softmax(ts, axis=-1)
        order = np.asarray(ti).reshape(NT, 2).astype(np.int64)
        w = np.asarray(tp, dtype=np.float32).reshape(NT, 2)
        return order, w
    except Exception:
        pass
    # numpy fallback (same math in f32)
    xf = np.asarray(x, dtype=np.float32).reshape(NT, C)
    logits = xf @ np.asarray(Wg, dtype=np.float32) + np.asarray(
        bg, dtype=np.float32)
    m = logits.max(axis=1, keepdims=True)
    p = np.exp(logits - m)
    p /= p.sum(axis=1, keepdims=True)
    # top-2 with lowest-index tie-break, like jax.lax.top_k
    order = np.argsort(-p, axis=1, kind="stable")[:, :2]
    p1 = np.take_along_axis(p, order, axis=1)          # [NT, 2], p1 >= p2
    e = np.exp(p1 - p1[:, :1])
    w = e / e.sum(axis=1, keepdims=True)               # renormalized weights
    return order, w.astype(np.float32)


def run(x, W1, b1, W2, b2, Wg, bg, trace=False, tmpdir=None):
    order, w = _route(x, Wg, bg)
    xb = np.asarray(x, dtype=np.float32).reshape(NT, C).astype(BF16)

    ids = []
    for e in range(E):
        sel = np.nonzero((order[:, 0] == e) | (order[:, 1] == e))[0]
        ids.append(sel)
    cap = max(P, max(len(s) for s in ids))

    nc = _get_nc(cap)
    in_maps = []
    for e in range(E):
        xg = np.zeros((cap, C), dtype=BF16)
        xg[:len(ids[e])] = xb[ids[e]]
        in_maps.append({
            "xgt": np.ascontiguousarray(xg.T),
            "w1": np.ascontiguousarray(W1[e]).astype(BF16),
            "b1c": np.ascontiguousarray(
                np.asarray(b1[e], np.float32).reshape(KH, P).T),
            "w2": np.ascontiguousarray(W2[e]).astype(BF16),
        })

    res = bu.run_bass_kernel_spmd(nc, in_maps, list(range(E)), trace=trace,
                                  tmpdir=tmpdir)

    # host unshard/combine: scatter expert rows back with gate weights
    acc = np.zeros((NT, C), dtype=np.float32)
    wsel = np.zeros((NT, E), dtype=np.float32)
    np.put_along_axis(wsel, order, w, axis=1)
    for e in range(E):
        ye = res.results[e]["out"][:len(ids[e])].astype(np.float32)
        acc[ids[e]] += wsel[ids[e], e][:, None] * ye
    b2f = np.asarray(b2, dtype=np.float32)
    acc += w[:, 0:1] * b2f[order[:, 0]] + w[:, 1:2] * b2f[order[:, 1]]
    return acc.reshape(B, T, C), res


def kernel(x, W1, b1, W2, b2, Wg, bg):
    out, _ = run(x, W1, b1, W2, b2, Wg, bg)
    return out


# revision 17
# speedup vs baseline: 1.0172x; 1.0172x over previous
"""MoE FFN (expert-parallel, host-routed) Trainium2 kernel.

Strategy: the gate depends only on inputs (x, Wg, bg), so routing is
part of the sharding decision: the host computes top-2 routing, compacts
each expert's tokens into a padded [CAP, C] block, and core e runs a
dense FFN (relu(x@W1+b1)@W2, bf16 matmuls / f32 accumulate) over expert
e's block. The host unshard step scatters each expert's rows back to
token order with the renormalized gate weights and adds the w-weighted
b2 terms (a 2-term axpy per token — the "psum" combine).

Device kernel layout per 256-token chunk (h-outer, W1/W2 interleaved so
the PE never idles): ps1[h] = sum_k W1[k,h].T @ xT[k]  (8 matmuls,
N=256) -> scalar-engine relu+b1 -> ht[h] bf16 -> ps2[tt,cc] +=
ht[h,tt].T @ W2[h,cc] (4 matmuls, N=512, accumulated over all 32 h).
W1/W2 stay SBUF-resident (128 KB/partition); PSUM: 4 banks ps2 + 2 ps1.
"""
import sys

sys.path.insert(0, "/opt/trn_rl_repo")

import numpy as np
import ml_dtypes

import bass_rust
import concourse.bass as bass
import concourse.mybir as mybir
import concourse.bass_utils as bu
from concourse.tile import TileContext

BF16 = ml_dtypes.bfloat16

B, T, C, E, H = 4, 2048, 1024, 8, 4096
NT = B * T          # 8192 tokens
P = 128
KC = C // P         # 8 k-tiles over C
KH = H // P         # 32 k-tiles over H

F32 = mybir.dt.float32
BF = mybir.dt.bfloat16
Relu = mybir.ActivationFunctionType.Relu


def _split_excess_waits(nc):
    """walrus codegen allows 1 sem-wait per instruction (2 on
    EventSemaphore). Move excess waits onto same-engine EventSemaphore
    insts placed just before (engine program order preserves semantics)."""
    for f in nc.m.functions:
        for bb in f.blocks:
            new = []
            changed = False
            for inst in bb.instructions:
                si = inst.sync_info
                cap = 2 if isinstance(inst, mybir.InstEventSemaphore) else 1
                if si is not None and len(si.on_wait) > cap:
                    waits = list(si.on_wait)
                    extra, keep = waits[:-cap], waits[-cap:]
                    for i in range(0, len(extra), 2):
                        w = mybir.InstEventSemaphore(
                            name=f"{inst.name}_presem{i}", ins=[], outs=[])
                        w.engine = inst.engine
                        w.sync_info = bass_rust.SyncInfo(
                            on_wait=extra[i:i + 2], on_update=[])
                        new.append(w)
                        changed = True
                    inst.sync_info = bass_rust.SyncInfo(
                        on_wait=keep, on_update=list(si.on_update))
                new.append(inst)
            if changed:
                bb.instructions = new


def _build_ffn(cap):
    """Dense FFN over cap pre-gathered tokens: y = relu(xT.T@W1 + b1)@W2."""
    nc = bass.Bass()
    xgt = nc.declare_dram_parameter("xgt", [C, cap], BF, isOutput=False)
    w1 = nc.declare_dram_parameter("w1", [C, H], BF, isOutput=False)
    b1c = nc.declare_dram_parameter("b1c", [P, KH], F32, isOutput=False)
    w2 = nc.declare_dram_parameter("w2", [H, C], BF, isOutput=False)
    out = nc.declare_dram_parameter("out", [cap, C], F32, isOutput=True)

    # chunk sizes: 256-token chunks with one (possibly ragged) tail
    sizes = []
    left = cap
    while left > 0:
        s = 256 if left >= 256 else left
        sizes.append(s)
        left -= s

    HH = H // 2  # W1 loaded as h-halves so chunk 0 starts sooner

    with TileContext(nc) as tc:
        with tc.tile_pool(name="wpool", bufs=1) as wpool, \
             tc.tile_pool(name="xpool", bufs=3) as xpool, \
             tc.tile_pool(name="hpool", bufs=21) as hpool, \
             tc.tile_pool(name="ypool", bufs=4) as ypool, \
             tc.tile_pool(name="ps1", bufs=3, space="PSUM") as ps1p, \
             tc.tile_pool(name="psd", bufs=1, space="PSUM") as psdp, \
             tc.tile_pool(name="ps2", bufs=1, space="PSUM") as ps2p:

            # DMA plan: each DGE queue sustains only ~160-200 GB/s, so split
            # every weight class across TWO queues ordered by first
            # consumption: W1 h-first-half, W1 h-second-half, then W2 in h
            # order. The queues are SP (sync) and Pool (gpsimd/SWDGE) — NOT
            # the ACT queue: DMA issues block the ACT sequencer ahead of the
            # relus emitted later on that same engine. ACT only carries the
            # tiny per-chunk x prefetches.
            b1c_sb = wpool.tile([P, KH], F32, tag="b1c")
            nc.sync.dma_start(out=b1c_sb[:], in_=b1c[:])
            xts_t = {}
            xgt_r = xgt.rearrange("(k p) t -> p k t", p=P)
            xts_t[0] = xpool.tile([P, KC, sizes[0]], BF,
                                  tag=f"xts{sizes[0]}", name="xts")
            nc.scalar.dma_start(out=xts_t[0][:], in_=xgt_r[:, :, 0:sizes[0]])

            def eng(i):
                return nc.sync if i % 2 == 0 else nc.gpsimd

            w1a_sb, w1b_sb = [], []
            for k in range(KC):
                ta = wpool.tile([P, HH], BF, tag=f"w1a{k}")
                eng(k).dma_start(out=ta[:], in_=w1[k * P:(k + 1) * P, 0:HH])
                w1a_sb.append(ta)
            for k in range(KC):
                tb = wpool.tile([P, HH], BF, tag=f"w1b{k}")
                eng(k).dma_start(out=tb[:], in_=w1[k * P:(k + 1) * P, HH:H])
                w1b_sb.append(tb)
            w2_sb = []
            for h in range(KH):
                t = wpool.tile([P, C], BF, tag=f"w2k{h}")
                eng(h).dma_start(out=t[:], in_=w2[h * P:(h + 1) * P, :])
                w2_sb.append(t)

            def w1_slice(k, h):
                if h < KH // 2:
                    return w1a_sb[k][:, h * P:(h + 1) * P]
                return w1b_sb[k][:, (h - KH // 2) * P:(h - KH // 2 + 1) * P]

            # HAM pre-warm: one long accumulation group of dummy matmuls
            # (no intermediate PSUM reads -> pure back-to-back PE stream)
            # keeps the PE busy from ~8us so the clock gate opens
            # (1.2 -> 2.4 GHz) before the real work arrives.
            NDUM = 44
            dsb = wpool.tile([P, 512], BF, tag="dummy")
            nc.vector.memset(dsb[:], 0.0)
            dps = psdp.tile([P, 512], F32, tag="dps", name="dps")
            for i in range(NDUM):
                nc.tensor.matmul(out=dps[:], lhsT=dsb[:, 0:P], rhs=dsb[:],
                                 start=(i == 0), stop=(i == NDUM - 1))

            c0 = 0
            for ci, tok in enumerate(sizes):
                ntt = -(-tok // P)
                # prefetch next chunk's x^T on the scalar queue
                if ci + 1 < len(sizes):
                    nt_ = sizes[ci + 1]
                    xts_t[ci + 1] = xpool.tile([P, KC, nt_], BF,
                                               tag=f"xts{nt_}", name="xts")
                    nc.scalar.dma_start(
                        out=xts_t[ci + 1][:],
                        in_=xgt_r[:, :, c0 + tok:c0 + tok + nt_])
                xts = xts_t.pop(ci)

                ps2 = [[ps2p.tile([P, 512], F32, tag=f"ps2_{tt}_{cc}",
                                  name=f"ps2_{tt}_{cc}")
                        for cc in range(2)] for tt in range(ntt)]
                ht = {}

                def w2_group(h):
                    for tt in range(ntt):
                        mm = min(P, tok - tt * P)
                        for cc in range(2):
                            nc.tensor.matmul(
                                out=ps2[tt][cc][0:mm, :],
                                lhsT=ht[h][:, tt * P:tt * P + mm],
                                rhs=w2_sb[h][:, cc * 512:(cc + 1) * 512],
                                start=(h == 0), stop=(h == KH - 1))

                for h in range(KH):
                    ps1 = ps1p.tile([P, 256], F32, tag="ps1",
                                    name="ps1")[:, 0:tok]
                    for k in range(KC):
                        nc.tensor.matmul(
                            out=ps1,
                            lhsT=w1_slice(k, h),
                            rhs=xts[:, k, :],
                            start=(k == 0), stop=(k == KC - 1))
                    htt = hpool.tile([P, 256], BF, tag="ht",
                                     name="ht")[:, 0:tok]
                    nc.scalar.activation(htt, ps1, Relu,
                                         bias=b1c_sb[:, h:h + 1])
                    ht[h] = htt
                    # W2 runs behind W1 so the relu latency is hidden by PE
                    # work; chunk 0 uses a deep lag so its W2 groups are
                    # reached only after the w2 h-tiles have streamed in.
                    lag = 18 if ci == 0 else 2
                    if h >= lag:
                        w2_group(h - lag)
                for h in range(KH - lag, KH):
                    w2_group(h)

                for tt in range(ntt):
                    mm = min(P, tok - tt * P)
                    y = ypool.tile([P, C], F32, tag="y", name="y")
                    # split the PSUM->SBUF drain across DVE and ACT so both
                    # halves land before the next chunk's first W2 group
                    nc.vector.tensor_copy(out=y[0:mm, 0:512],
                                          in_=ps2[tt][0][0:mm, :])
                    nc.scalar.activation(
                        y[0:mm, 512:1024], ps2[tt][1][0:mm, :],
                        mybir.ActivationFunctionType.Copy)
                    nc.sync.dma_start(
                        out=out[c0 + tt * P:c0 + tt * P + mm, :],
                        in_=y[0:mm, :])
                c0 += tok

    import os
    if os.environ.get("NOSPLIT", "0") != "1":
        _split_excess_waits(nc)
    return nc


_NC_CACHE = {}


def _get_nc(cap):
    if cap not in _NC_CACHE:
        _NC_CACHE[cap] = _build_ffn(cap)
    return _NC_CACHE[cap]


def _route(x, Wg, bg):
    """Host top-2 routing with the exact ops the reference uses, so the
    selection bit-matches the oracle on whatever jax backend is active."""
    try:
        import jax
        import jax.numpy as jnp

        gs = jax.nn.softmax(
            jnp.einsum("btc,ce->bte", jnp.asarray(x), jnp.asarray(Wg))
            + jnp.asarray(bg), axis=-1)
        ts, ti = jax.lax.top_k(gs, 2)
        tp = jax.nn.softmax(ts, axis=-1)
        order = np.asarray(ti).reshape(NT, 2).astype(np.int64)
        w = np.asarray(tp, dtype=np.float32).reshape(NT, 2)
        return order, w
    except Exception:
        pass
    # numpy fallback (same math in f32)
    xf = np.asarray(x, dtype=np.float32).reshape(NT, C)
    logits = xf @ np.asarray(Wg, dtype=np.float32) + np.asarray(
        bg, dtype=np.float32)
    m = logits.max(axis=1, keepdims=True)
    p = np.exp(logits - m)
    p /= p.sum(axis=1, keepdims=True)
    # top-2 with lowest-index tie-break, like jax.lax.top_k
    order = np.argsort(-p, axis=1, kind="stable")[:, :2]
    p1 = np.take_along_axis(p, order, axis=1)          # [NT, 2], p1 >= p2
    e = np.exp(p1 - p1[:, :1])
    w = e / e.sum(axis=1, keepdims=True)               # renormalized weights
    return order, w.astype(np.float32)


def run(x, W1, b1, W2, b2, Wg, bg, trace=False, tmpdir=None):
    order, w = _route(x, Wg, bg)
    xb = np.asarray(x, dtype=np.float32).reshape(NT, C).astype(BF16)

    ids = []
    for e in range(E):
        sel = np.nonzero((order[:, 0] == e) | (order[:, 1] == e))[0]
        ids.append(sel)
    cap = max(P, max(len(s) for s in ids))

    nc = _get_nc(cap)
    in_maps = []
    for e in range(E):
        xg = np.zeros((cap, C), dtype=BF16)
        xg[:len(ids[e])] = xb[ids[e]]
        in_maps.append({
            "xgt": np.ascontiguousarray(xg.T),
            "w1": np.ascontiguousarray(W1[e]).astype(BF16),
            "b1c": np.ascontiguousarray(
                np.asarray(b1[e], np.float32).reshape(KH, P).T),
            "w2": np.ascontiguousarray(W2[e]).astype(BF16),
        })

    res = bu.run_bass_kernel_spmd(nc, in_maps, list(range(E)), trace=trace,
                                  tmpdir=tmpdir)

    # host unshard/combine: scatter expert rows back with gate weights
    acc = np.zeros((NT, C), dtype=np.float32)
    wsel = np.zeros((NT, E), dtype=np.float32)
    np.put_along_axis(wsel, order, w, axis=1)
    for e in range(E):
        ye = res.results[e]["out"][:len(ids[e])].astype(np.float32)
        acc[ids[e]] += wsel[ids[e], e][:, None] * ye
    b2f = np.asarray(b2, dtype=np.float32)
    acc += w[:, 0:1] * b2f[order[:, 0]] + w[:, 1:2] * b2f[order[:, 1]]
    return acc.reshape(B, T, C), res


def kernel(x, W1, b1, W2, b2, Wg, bg):
    out, _ = run(x, W1, b1, W2, b2, Wg, bg)
    return out


# revision 18
# speedup vs baseline: 1.0486x; 1.0309x over previous
"""MoE FFN (expert-parallel, host-routed) Trainium2 kernel.

Strategy: the gate depends only on inputs (x, Wg, bg), so routing is
part of the sharding decision: the host computes top-2 routing, compacts
each expert's tokens into a padded [CAP, C] block, and core e runs a
dense FFN (relu(x@W1+b1)@W2, bf16 matmuls / f32 accumulate) over expert
e's block. The host unshard step scatters each expert's rows back to
token order with the renormalized gate weights and adds the w-weighted
b2 terms (a 2-term axpy per token — the "psum" combine).

Device kernel layout per 256-token chunk (h-outer, W1/W2 interleaved so
the PE never idles): ps1[h] = sum_k W1[k,h].T @ xT[k]  (8 matmuls,
N=256) -> scalar-engine relu+b1 -> ht[h] bf16 -> ps2[tt,cc] +=
ht[h,tt].T @ W2[h,cc] (4 matmuls, N=512, accumulated over all 32 h).
W1/W2 stay SBUF-resident (128 KB/partition); PSUM: 4 banks ps2 + 2 ps1.
"""
import sys

sys.path.insert(0, "/opt/trn_rl_repo")

import numpy as np
import ml_dtypes

import bass_rust
import concourse.bass as bass
import concourse.mybir as mybir
import concourse.bass_utils as bu
from concourse.tile import TileContext

BF16 = ml_dtypes.bfloat16

B, T, C, E, H = 4, 2048, 1024, 8, 4096
NT = B * T          # 8192 tokens
P = 128
KC = C // P         # 8 k-tiles over C
KH = H // P         # 32 k-tiles over H

F32 = mybir.dt.float32
BF = mybir.dt.bfloat16
Relu = mybir.ActivationFunctionType.Relu


def _split_excess_waits(nc):
    """walrus codegen allows 1 sem-wait per instruction (2 on
    EventSemaphore). Move excess waits onto same-engine EventSemaphore
    insts placed just before (engine program order preserves semantics)."""
    for f in nc.m.functions:
        for bb in f.blocks:
            new = []
            changed = False
            for inst in bb.instructions:
                si = inst.sync_info
                cap = 2 if isinstance(inst, mybir.InstEventSemaphore) else 1
                if si is not None and len(si.on_wait) > cap:
                    waits = list(si.on_wait)
                    extra, keep = waits[:-cap], waits[-cap:]
                    for i in range(0, len(extra), 2):
                        w = mybir.InstEventSemaphore(
                            name=f"{inst.name}_presem{i}", ins=[], outs=[])
                        w.engine = inst.engine
                        w.sync_info = bass_rust.SyncInfo(
                            on_wait=extra[i:i + 2], on_update=[])
                        new.append(w)
                        changed = True
                    inst.sync_info = bass_rust.SyncInfo(
                        on_wait=keep, on_update=list(si.on_update))
                new.append(inst)
            if changed:
                bb.instructions = new


def _build_ffn(cap):
    """Dense FFN over cap pre-gathered tokens: y = relu(xT.T@W1 + b1)@W2."""
    nc = bass.Bass()
    xgt = nc.declare_dram_parameter("xgt", [C, cap], BF, isOutput=False)
    w1 = nc.declare_dram_parameter("w1", [C, H], BF, isOutput=False)
    b1c = nc.declare_dram_parameter("b1c", [P, KH], F32, isOutput=False)
    w2 = nc.declare_dram_parameter("w2", [H, C], BF, isOutput=False)
    out = nc.declare_dram_parameter("out", [cap, C], F32, isOutput=True)

    # chunk sizes: 256-token chunks with one (possibly ragged) tail
    sizes = []
    left = cap
    while left > 0:
        s = 256 if left >= 256 else left
        sizes.append(s)
        left -= s

    HH = H // 2  # W1 loaded as h-halves so chunk 0 starts sooner

    with TileContext(nc) as tc:
        with tc.tile_pool(name="wpool", bufs=1) as wpool, \
             tc.tile_pool(name="xpool", bufs=1) as xpool, \
             tc.tile_pool(name="hpool", bufs=21) as hpool, \
             tc.tile_pool(name="ypool", bufs=4) as ypool, \
             tc.tile_pool(name="ps1", bufs=3, space="PSUM") as ps1p, \
             tc.tile_pool(name="psd", bufs=1, space="PSUM") as psdp, \
             tc.tile_pool(name="ps2", bufs=1, space="PSUM") as ps2p:

            # DMA plan: each DGE queue sustains only ~160-200 GB/s, so split
            # every weight class across TWO queues ordered by first
            # consumption: W1 h-first-half, W1 h-second-half, then W2 in h
            # order. The queues are SP (sync) and Pool (gpsimd/SWDGE) — NOT
            # the ACT queue: DMA issues block the ACT sequencer ahead of the
            # relus emitted later on that same engine. ACT only carries the
            # tiny per-chunk x prefetches.
            b1c_sb = wpool.tile([P, KH], F32, tag="b1c")
            nc.sync.dma_start(out=b1c_sb[:], in_=b1c[:])
            xts_t = {}
            xgt_r = xgt.rearrange("(k p) t -> p k t", p=P)
            cc0 = 0
            for ci, tok in enumerate(sizes):
                xts_t[ci] = xpool.tile([P, KC, tok], BF,
                                       tag=f"xts_c{ci}", name="xts")
                nc.scalar.dma_start(out=xts_t[ci][:],
                                    in_=xgt_r[:, :, cc0:cc0 + tok])
                cc0 += tok

            def eng(i):
                return nc.sync

            w1a_sb, w1b_sb = [], []
            for k in range(KC):
                ta = wpool.tile([P, HH], BF, tag=f"w1a{k}")
                eng(k).dma_start(out=ta[:], in_=w1[k * P:(k + 1) * P, 0:HH])
                w1a_sb.append(ta)
            for k in range(KC):
                tb = wpool.tile([P, HH], BF, tag=f"w1b{k}")
                eng(k).dma_start(out=tb[:], in_=w1[k * P:(k + 1) * P, HH:H])
                w1b_sb.append(tb)
            w2_sb = []
            for h in range(KH):
                t = wpool.tile([P, C], BF, tag=f"w2k{h}")
                eng(h).dma_start(out=t[:], in_=w2[h * P:(h + 1) * P, :])
                w2_sb.append(t)

            def w1_slice(k, h):
                if h < KH // 2:
                    return w1a_sb[k][:, h * P:(h + 1) * P]
                return w1b_sb[k][:, (h - KH // 2) * P:(h - KH // 2 + 1) * P]

            # HAM pre-warm: one long accumulation group of dummy matmuls
            # (no intermediate PSUM reads -> pure back-to-back PE stream)
            # keeps the PE busy from ~8us so the clock gate opens
            # (1.2 -> 2.4 GHz) before the real work arrives.
            NDUM = 44
            dsb = wpool.tile([P, 512], BF, tag="dummy")
            nc.vector.memset(dsb[:], 0.0)
            dps = psdp.tile([P, 512], F32, tag="dps", name="dps")
            for i in range(NDUM):
                nc.tensor.matmul(out=dps[:], lhsT=dsb[:, 0:P], rhs=dsb[:],
                                 start=(i == 0), stop=(i == NDUM - 1))

            c0 = 0
            for ci, tok in enumerate(sizes):
                ntt = -(-tok // P)
                xts = xts_t.pop(ci)

                ps2 = [[ps2p.tile([P, 512], F32, tag=f"ps2_{tt}_{cc}",
                                  name=f"ps2_{tt}_{cc}")
                        for cc in range(2)] for tt in range(ntt)]
                ht = {}

                def w2_group(h):
                    for tt in range(ntt):
                        mm = min(P, tok - tt * P)
                        for cc in range(2):
                            nc.tensor.matmul(
                                out=ps2[tt][cc][0:mm, :],
                                lhsT=ht[h][:, tt * P:tt * P + mm],
                                rhs=w2_sb[h][:, cc * 512:(cc + 1) * 512],
                                start=(h == 0), stop=(h == KH - 1))

                for h in range(KH):
                    ps1 = ps1p.tile([P, 256], F32, tag="ps1",
                                    name="ps1")[:, 0:tok]
                    for k in range(KC):
                        nc.tensor.matmul(
                            out=ps1,
                            lhsT=w1_slice(k, h),
                            rhs=xts[:, k, :],
                            start=(k == 0), stop=(k == KC - 1))
                    htt = hpool.tile([P, 256], BF, tag="ht",
                                     name="ht")[:, 0:tok]
                    nc.scalar.activation(htt, ps1, Relu,
                                         bias=b1c_sb[:, h:h + 1])
                    ht[h] = htt
                    # W2 runs behind W1 so the relu latency is hidden by PE
                    # work; chunk 0 uses a deep lag so its W2 groups are
                    # reached only after the w2 h-tiles have streamed in.
                    lag = 18 if ci == 0 else 2
                    if h >= lag:
                        w2_group(h - lag)
                for h in range(KH - lag, KH):
                    w2_group(h)

                for tt in range(ntt):
                    mm = min(P, tok - tt * P)
                    y = ypool.tile([P, C], F32, tag="y", name="y")
                    # split the PSUM->SBUF drain across DVE and ACT so both
                    # halves land before the next chunk's first W2 group
                    nc.vector.tensor_copy(out=y[0:mm, 0:512],
                                          in_=ps2[tt][0][0:mm, :])
                    nc.scalar.activation(
                        y[0:mm, 512:1024], ps2[tt][1][0:mm, :],
                        mybir.ActivationFunctionType.Copy)
                    nc.gpsimd.dma_start(
                        out=out[c0 + tt * P:c0 + tt * P + mm, :],
                        in_=y[0:mm, :])
                c0 += tok

    import os
    if os.environ.get("NOSPLIT", "0") != "1":
        _split_excess_waits(nc)
    return nc


_NC_CACHE = {}


def _get_nc(cap):
    if cap not in _NC_CACHE:
        _NC_CACHE[cap] = _build_ffn(cap)
    return _NC_CACHE[cap]


def _route(x, Wg, bg):
    """Host top-2 routing with the exact ops the reference uses, so the
    selection bit-matches the oracle on whatever jax backend is active."""
    try:
        import jax
        import jax.numpy as jnp

        gs = jax.nn.softmax(
            jnp.einsum("btc,ce->bte", jnp.asarray(x), jnp.asarray(Wg))
            + jnp.asarray(bg), axis=-1)
        ts, ti = jax.lax.top_k(gs, 2)
        tp = jax.nn.softmax(ts, axis=-1)
        order = np.asarray(ti).reshape(NT, 2).astype(np.int64)
        w = np.asarray(tp, dtype=np.float32).reshape(NT, 2)
        return order, w
    except Exception:
        pass
    # numpy fallback (same math in f32)
    xf = np.asarray(x, dtype=np.float32).reshape(NT, C)
    logits = xf @ np.asarray(Wg, dtype=np.float32) + np.asarray(
        bg, dtype=np.float32)
    m = logits.max(axis=1, keepdims=True)
    p = np.exp(logits - m)
    p /= p.sum(axis=1, keepdims=True)
    # top-2 with lowest-index tie-break, like jax.lax.top_k
    order = np.argsort(-p, axis=1, kind="stable")[:, :2]
    p1 = np.take_along_axis(p, order, axis=1)          # [NT, 2], p1 >= p2
    e = np.exp(p1 - p1[:, :1])
    w = e / e.sum(axis=1, keepdims=True)               # renormalized weights
    return order, w.astype(np.float32)


def run(x, W1, b1, W2, b2, Wg, bg, trace=False, tmpdir=None):
    order, w = _route(x, Wg, bg)
    xb = np.asarray(x, dtype=np.float32).reshape(NT, C).astype(BF16)

    ids = []
    for e in range(E):
        sel = np.nonzero((order[:, 0] == e) | (order[:, 1] == e))[0]
        ids.append(sel)
    cap = max(P, max(len(s) for s in ids))

    nc = _get_nc(cap)
    in_maps = []
    for e in range(E):
        xg = np.zeros((cap, C), dtype=BF16)
        xg[:len(ids[e])] = xb[ids[e]]
        in_maps.append({
            "xgt": np.ascontiguousarray(xg.T),
            "w1": np.ascontiguousarray(W1[e]).astype(BF16),
            "b1c": np.ascontiguousarray(
                np.asarray(b1[e], np.float32).reshape(KH, P).T),
            "w2": np.ascontiguousarray(W2[e]).astype(BF16),
        })

    res = bu.run_bass_kernel_spmd(nc, in_maps, list(range(E)), trace=trace,
                                  tmpdir=tmpdir)

    # host unshard/combine: scatter expert rows back with gate weights
    acc = np.zeros((NT, C), dtype=np.float32)
    wsel = np.zeros((NT, E), dtype=np.float32)
    np.put_along_axis(wsel, order, w, axis=1)
    for e in range(E):
        ye = res.results[e]["out"][:len(ids[e])].astype(np.float32)
        acc[ids[e]] += wsel[ids[e], e][:, None] * ye
    b2f = np.asarray(b2, dtype=np.float32)
    acc += w[:, 0:1] * b2f[order[:, 0]] + w[:, 1:2] * b2f[order[:, 1]]
    return acc.reshape(B, T, C), res


def kernel(x, W1, b1, W2, b2, Wg, bg):
    out, _ = run(x, W1, b1, W2, b2, Wg, bg)
    return out


# revision 19
# speedup vs baseline: 1.0616x; 1.0124x over previous
"""MoE FFN (expert-parallel, host-routed) Trainium2 kernel.

Strategy: the gate depends only on inputs (x, Wg, bg), so routing is
part of the sharding decision: the host computes top-2 routing, compacts
each expert's tokens into a padded [CAP, C] block, and core e runs a
dense FFN (relu(x@W1+b1)@W2, bf16 matmuls / f32 accumulate) over expert
e's block. The host unshard step scatters each expert's rows back to
token order with the renormalized gate weights and adds the w-weighted
b2 terms (a 2-term axpy per token — the "psum" combine).

Device kernel layout per 256-token chunk (h-outer, W1/W2 interleaved so
the PE never idles): ps1[h] = sum_k W1[k,h].T @ xT[k]  (8 matmuls,
N=256) -> scalar-engine relu+b1 -> ht[h] bf16 -> ps2[tt,cc] +=
ht[h,tt].T @ W2[h,cc] (4 matmuls, N=512, accumulated over all 32 h).
W1/W2 stay SBUF-resident (128 KB/partition); PSUM: 4 banks ps2 + 2 ps1.
"""
import sys

sys.path.insert(0, "/opt/trn_rl_repo")

import numpy as np
import ml_dtypes

import bass_rust
import concourse.bass as bass
import concourse.mybir as mybir
import concourse.bass_utils as bu
from concourse.tile import TileContext

BF16 = ml_dtypes.bfloat16

B, T, C, E, H = 4, 2048, 1024, 8, 4096
NT = B * T          # 8192 tokens
P = 128
KC = C // P         # 8 k-tiles over C
KH = H // P         # 32 k-tiles over H

F32 = mybir.dt.float32
BF = mybir.dt.bfloat16
Relu = mybir.ActivationFunctionType.Relu


def _split_excess_waits(nc):
    """walrus codegen allows 1 sem-wait per instruction (2 on
    EventSemaphore). Move excess waits onto same-engine EventSemaphore
    insts placed just before (engine program order preserves semantics)."""
    for f in nc.m.functions:
        for bb in f.blocks:
            new = []
            changed = False
            for inst in bb.instructions:
                si = inst.sync_info
                cap = 2 if isinstance(inst, mybir.InstEventSemaphore) else 1
                if si is not None and len(si.on_wait) > cap:
                    waits = list(si.on_wait)
                    extra, keep = waits[:-cap], waits[-cap:]
                    for i in range(0, len(extra), 2):
                        w = mybir.InstEventSemaphore(
                            name=f"{inst.name}_presem{i}", ins=[], outs=[])
                        w.engine = inst.engine
                        w.sync_info = bass_rust.SyncInfo(
                            on_wait=extra[i:i + 2], on_update=[])
                        new.append(w)
                        changed = True
                    inst.sync_info = bass_rust.SyncInfo(
                        on_wait=keep, on_update=list(si.on_update))
                new.append(inst)
            if changed:
                bb.instructions = new


def _build_ffn(cap):
    """Dense FFN over cap pre-gathered tokens: y = relu(xT.T@W1 + b1)@W2."""
    nc = bass.Bass()
    xgt = nc.declare_dram_parameter("xgt", [C, cap], BF, isOutput=False)
    w1 = nc.declare_dram_parameter("w1", [C, H], BF, isOutput=False)
    b1c = nc.declare_dram_parameter("b1c", [P, KH], F32, isOutput=False)
    w2 = nc.declare_dram_parameter("w2", [H, C], BF, isOutput=False)
    out = nc.declare_dram_parameter("out", [cap, C], F32, isOutput=True)

    # chunk sizes: 256-token chunks with one (possibly ragged) tail
    sizes = []
    left = cap
    while left > 0:
        s = 256 if left >= 256 else left
        sizes.append(s)
        left -= s

    HH = H // 2  # W1 loaded as h-halves so chunk 0 starts sooner

    with TileContext(nc) as tc:
        with tc.tile_pool(name="wpool", bufs=1) as wpool, \
             tc.tile_pool(name="xpool", bufs=1) as xpool, \
             tc.tile_pool(name="hpool", bufs=21) as hpool, \
             tc.tile_pool(name="ypool", bufs=4) as ypool, \
             tc.tile_pool(name="ps1", bufs=3, space="PSUM") as ps1p, \
             tc.tile_pool(name="psd", bufs=1, space="PSUM") as psdp, \
             tc.tile_pool(name="ps2", bufs=1, space="PSUM") as ps2p:

            # DMA plan: each DGE queue sustains only ~160-200 GB/s, so split
            # every weight class across TWO queues ordered by first
            # consumption: W1 h-first-half, W1 h-second-half, then W2 in h
            # order. The queues are SP (sync) and Pool (gpsimd/SWDGE) — NOT
            # the ACT queue: DMA issues block the ACT sequencer ahead of the
            # relus emitted later on that same engine. ACT only carries the
            # tiny per-chunk x prefetches.
            b1c_sb = wpool.tile([P, KH], F32, tag="b1c")
            nc.sync.dma_start(out=b1c_sb[:], in_=b1c[:])
            xts_t = {}
            xgt_r = xgt.rearrange("(k p) t -> p k t", p=P)
            xoff = [0]
            for tok in sizes:
                xoff.append(xoff[-1] + tok)

            def load_xts(ci):
                tok = sizes[ci]
                xts_t[ci] = xpool.tile([P, KC, tok], BF,
                                       tag=f"xts_c{ci}", name="xts")
                nc.sync.dma_start(out=xts_t[ci][:],
                                  in_=xgt_r[:, :, xoff[ci]:xoff[ci] + tok])

            load_xts(0)

            def eng(i):
                return nc.sync

            w1a_sb, w1b_sb = [], []
            for k in range(KC):
                ta = wpool.tile([P, HH], BF, tag=f"w1a{k}")
                eng(k).dma_start(out=ta[:], in_=w1[k * P:(k + 1) * P, 0:HH])
                w1a_sb.append(ta)
            for ci in range(1, min(3, len(sizes))):
                load_xts(ci)
            for k in range(KC):
                tb = wpool.tile([P, HH], BF, tag=f"w1b{k}")
                eng(k).dma_start(out=tb[:], in_=w1[k * P:(k + 1) * P, HH:H])
                w1b_sb.append(tb)
            for ci in range(3, len(sizes)):
                load_xts(ci)
            w2_sb = []
            for h in range(KH):
                t = wpool.tile([P, C], BF, tag=f"w2k{h}")
                eng(h).dma_start(out=t[:], in_=w2[h * P:(h + 1) * P, :])
                w2_sb.append(t)

            def w1_slice(k, h):
                if h < KH // 2:
                    return w1a_sb[k][:, h * P:(h + 1) * P]
                return w1b_sb[k][:, (h - KH // 2) * P:(h - KH // 2 + 1) * P]

            # HAM pre-warm: one long accumulation group of dummy matmuls
            # (no intermediate PSUM reads -> pure back-to-back PE stream)
            # keeps the PE busy from ~8us so the clock gate opens
            # (1.2 -> 2.4 GHz) before the real work arrives.
            NDUM = 44
            dsb = wpool.tile([P, 512], BF, tag="dummy")
            nc.vector.memset(dsb[:], 0.0)
            dps = psdp.tile([P, 512], F32, tag="dps", name="dps")
            for i in range(NDUM):
                nc.tensor.matmul(out=dps[:], lhsT=dsb[:, 0:P], rhs=dsb[:],
                                 start=(i == 0), stop=(i == NDUM - 1))

            c0 = 0
            for ci, tok in enumerate(sizes):
                ntt = -(-tok // P)
                xts = xts_t.pop(ci)

                ps2 = [[ps2p.tile([P, 512], F32, tag=f"ps2_{tt}_{cc}",
                                  name=f"ps2_{tt}_{cc}")
                        for cc in range(2)] for tt in range(ntt)]
                ht = {}

                def w2_group(h):
                    for tt in range(ntt):
                        mm = min(P, tok - tt * P)
                        for cc in range(2):
                            nc.tensor.matmul(
                                out=ps2[tt][cc][0:mm, :],
                                lhsT=ht[h][:, tt * P:tt * P + mm],
                                rhs=w2_sb[h][:, cc * 512:(cc + 1) * 512],
                                start=(h == 0), stop=(h == KH - 1))

                for h in range(KH):
                    ps1 = ps1p.tile([P, 256], F32, tag="ps1",
                                    name="ps1")[:, 0:tok]
                    for k in range(KC):
                        nc.tensor.matmul(
                            out=ps1,
                            lhsT=w1_slice(k, h),
                            rhs=xts[:, k, :],
                            start=(k == 0), stop=(k == KC - 1))
                    htt = hpool.tile([P, 256], BF, tag="ht",
                                     name="ht")[:, 0:tok]
                    nc.scalar.activation(htt, ps1, Relu,
                                         bias=b1c_sb[:, h:h + 1])
                    ht[h] = htt
                    # W2 runs behind W1 so the relu latency is hidden by PE
                    # work; chunk 0 uses a deep lag so its W2 groups are
                    # reached only after the w2 h-tiles have streamed in.
                    lag = 18 if ci == 0 else 2
                    if h >= lag:
                        w2_group(h - lag)
                for h in range(KH - lag, KH):
                    w2_group(h)

                for tt in range(ntt):
                    mm = min(P, tok - tt * P)
                    y = ypool.tile([P, C], F32, tag="y", name="y")
                    # split the PSUM->SBUF drain across DVE and ACT so both
                    # halves land before the next chunk's first W2 group
                    nc.vector.tensor_copy(out=y[0:mm, 0:512],
                                          in_=ps2[tt][0][0:mm, :])
                    nc.scalar.activation(
                        y[0:mm, 512:1024], ps2[tt][1][0:mm, :],
                        mybir.ActivationFunctionType.Copy)
                    nc.sync.dma_start(
                        out=out[c0 + tt * P:c0 + tt * P + mm, :],
                        in_=y[0:mm, :])
                c0 += tok

    import os
    if os.environ.get("NOSPLIT", "0") != "1":
        _split_excess_waits(nc)
    return nc


_NC_CACHE = {}


def _get_nc(cap):
    if cap not in _NC_CACHE:
        _NC_CACHE[cap] = _build_ffn(cap)
    return _NC_CACHE[cap]


def _route(x, Wg, bg):
    """Host top-2 routing with the exact ops the reference uses, so the
    selection bit-matches the oracle on whatever jax backend is active."""
    try:
        import jax
        import jax.numpy as jnp

        gs = jax.nn.softmax(
            jnp.einsum("btc,ce->bte", jnp.asarray(x), jnp.asarray(Wg))
            + jnp.asarray(bg), axis=-1)
        ts, ti = jax.lax.top_k(gs, 2)
        tp = jax.nn.softmax(ts, axis=-1)
        order = np.asarray(ti).reshape(NT, 2).astype(np.int64)
        w = np.asarray(tp, dtype=np.float32).reshape(NT, 2)
        return order, w
    except Exception:
        pass
    # numpy fallback (same math in f32)
    xf = np.asarray(x, dtype=np.float32).reshape(NT, C)
    logits = xf @ np.asarray(Wg, dtype=np.float32) + np.asarray(
        bg, dtype=np.float32)
    m = logits.max(axis=1, keepdims=True)
    p = np.exp(logits - m)
    p /= p.sum(axis=1, keepdims=True)
    # top-2 with lowest-index tie-break, like jax.lax.top_k
    order = np.argsort(-p, axis=1, kind="stable")[:, :2]
    p1 = np.take_along_axis(p, order, axis=1)          # [NT, 2], p1 >= p2
    e = np.exp(p1 - p1[:, :1])
    w = e / e.sum(axis=1, keepdims=True)               # renormalized weights
    return order, w.astype(np.float32)


def run(x, W1, b1, W2, b2, Wg, bg, trace=False, tmpdir=None):
    order, w = _route(x, Wg, bg)
    xb = np.asarray(x, dtype=np.float32).reshape(NT, C).astype(BF16)

    ids = []
    for e in range(E):
        sel = np.nonzero((order[:, 0] == e) | (order[:, 1] == e))[0]
        ids.append(sel)
    cap = max(P, max(len(s) for s in ids))

    nc = _get_nc(cap)
    in_maps = []
    for e in range(E):
        xg = np.zeros((cap, C), dtype=BF16)
        xg[:len(ids[e])] = xb[ids[e]]
        in_maps.append({
            "xgt": np.ascontiguousarray(xg.T),
            "w1": np.ascontiguousarray(W1[e]).astype(BF16),
            "b1c": np.ascontiguousarray(
                np.asarray(b1[e], np.float32).reshape(KH, P).T),
            "w2": np.ascontiguousarray(W2[e]).astype(BF16),
        })

    res = bu.run_bass_kernel_spmd(nc, in_maps, list(range(E)), trace=trace,
                                  tmpdir=tmpdir)

    # host unshard/combine: scatter expert rows back with gate weights
    acc = np.zeros((NT, C), dtype=np.float32)
    wsel = np.zeros((NT, E), dtype=np.float32)
    np.put_along_axis(wsel, order, w, axis=1)
    for e in range(E):
        ye = res.results[e]["out"][:len(ids[e])].astype(np.float32)
        acc[ids[e]] += wsel[ids[e], e][:, None] * ye
    b2f = np.asarray(b2, dtype=np.float32)
    acc += w[:, 0:1] * b2f[order[:, 0]] + w[:, 1:2] * b2f[order[:, 1]]
    return acc.reshape(B, T, C), res


def kernel(x, W1, b1, W2, b2, Wg, bg):
    out, _ = run(x, W1, b1, W2, b2, Wg, bg)
    return out
